# revision 14
# baseline (speedup 1.0000x reference)
"""Trainium2 Bass kernel for nn_AtomicFunction (pairwise-product quadratic form).

Reference math:
    out[b] = sum_k (w0[k]+w1[k]) * w0[k] * w1[k] * x[b, i_k] * x[b, j_k]
where (i_k, j_k) enumerate all C(256,2)=32640 pairs i<j (perm_mat) and
w = weight_inputs[0].  This is a quadratic form: with the [256,256] upper-
triangular coefficient matrix M[i_k, j_k] = (w0+w1)*w0*w1,
    out[b] = sum_j (x @ M)[b, j] * x[b, j]
M depends only on the small replicated weights, so it is built on host.

Sharding: data-parallel over B=1024 across 8 cores (128 rows each); M and
the ones-column are replicated.

The kernel works in transposed space (x is shipped as x^T per shard) so no
on-device transposes are needed and the result lands as a contiguous
[1, 128] row:
    tT[j, b] = sum_i M[i, j] * x[b, i]       (3 matmuls; M block-triangular,
                                              the zero block M[128:,:128] is
                                              never shipped or multiplied)
    p[j, b]  = tT[j, b] * xT[j, b]           (2 DVE elementwise muls)
    out[b]   = sum_j p[j, b]                 (ones-vector matmuls -> [1,128])

All operands ride in one [128, 641] f32 blob split across the two HWDGE
rings (scalar ring: x^T halves; sync ring: M blocks + ones) so each core
does exactly two input DMAs and one 512-byte output DMA.
Blob columns: xT0 | xT1 | M00 | M01 | M11 | ones.
"""

import os

import numpy as np

B = 1024
DIM = 256
NCORES = 8
ROWS = B // NCORES  # 128
P = 128
BLOB_W = 2 * P + 3 * P + 1  # 641

_state: dict = {}


def _lean_drain_and_barrier(self, tick_clock, wait_clock):
    """Lean TileContext tail for a single top-level context: drain with the
    full global-clock sem waits (proving every engine retired and the
    output DMA completed), ONE all-engine barrier so every engine has
    synced before the sem/DGE reset, then the reset.  The stock tail adds
    a second all-engine barrier after the reset, which only protects sem
    reuse by sibling tile contexts — this module has none.  Saves ~0.26us
    per launch and passes the CoreSim race detector's sem-clear rule."""
    from concourse.vector_clock import ScopedClock

    drain_inst = self.nc.sync.drain()
    wait_clock.add_sem_waits(drain_inst.ins, ScopedClock({None: tick_clock.global_clock}))
    self.nc.all_engine_barrier()
    popped = self.nc._tile_sem_poison_stack.pop()
    assert popped is self._sem_poison
    self.nc.clear_and_free_semaphores(list(self.sems.allocated().values()))


def _build_module(lean_tail=True):
    import concourse.mybir as mybir
    import concourse.tile as tile
    from concourse import bacc

    f32 = mybir.dt.float32
    nc = bacc.Bacc("TRN2", num_devices=NCORES)
    blob_d = nc.dram_tensor("blob", [P, BLOB_W], f32, kind="ExternalInput").ap()
    o_d = nc.dram_tensor("out", [1, ROWS], f32, kind="ExternalOutput").ap()

    if lean_tail:
        _orig_drain = tile.TileContext._drain_and_barrier
        tile.TileContext._drain_and_barrier = _lean_drain_and_barrier
        try:
            _emit_tile_body(nc, tile, f32, blob_d, o_d)
        finally:
            tile.TileContext._drain_and_barrier = _orig_drain
    else:
        _emit_tile_body(nc, tile, f32, blob_d, o_d)

    nc.compile()
    return nc


def _emit_tile_body(nc, tile, f32, blob_d, o_d):
    with tile.TileContext(nc) as tc:
        with (
            tc.tile_pool(name="sbuf", bufs=1) as pool,
            tc.tile_pool(name="psum", bufs=1, space="PSUM") as psum,
        ):
            blob = pool.tile([P, BLOB_W], f32)
            nc.scalar.dma_start(blob[:, 0:256], blob_d[:, 0:256])
            nc.sync.dma_start(blob[:, 256:BLOB_W], blob_d[:, 256:BLOB_W])
            xT0 = blob[:, 0:128]
            xT1 = blob[:, 128:256]
            M00 = blob[:, 256:384]
            M01 = blob[:, 384:512]
            M11 = blob[:, 512:640]
            ones = blob[:, 640:641]

            # tT = M^T-blocks @ xT, PSUM f32 accumulation
            tL = psum.tile([P, P], f32, tag="tL")
            nc.tensor.matmul(tL[:], M00, xT0, start=True, stop=True)
            tR = psum.tile([P, P], f32, tag="tR")
            nc.tensor.matmul(tR[:], M01, xT0, start=True, stop=False)
            nc.tensor.matmul(tR[:], M11, xT1, start=False, stop=True)

            # p = tT * xT elementwise
            p0 = pool.tile([P, P], f32, tag="p0")
            nc.vector.tensor_mul(out=p0[:], in0=tL[:], in1=xT0)
            p1 = pool.tile([P, P], f32, tag="p1")
            nc.vector.tensor_mul(out=p1[:], in0=tR[:], in1=xT1)

            # partition-dim reduction via ones-vector matmul -> [1, 128]
            op = psum.tile([1, P], f32, tag="op")
            nc.tensor.matmul(op[:], ones, p0[:], start=True, stop=False)
            nc.tensor.matmul(op[:], ones, p1[:], start=False, stop=True)
            resT = pool.tile([1, P], f32)
            nc.vector.tensor_copy(out=resT[:], in_=op[:])
            nc.sync.dma_start(o_d[:], resT[:])


def _get_nc():
    if "nc" not in _state:
        try:
            _state["nc"] = _build_module(lean_tail=True)
        except Exception:
            # the lean tail touches TileContext internals; fall back to the
            # stock drain+barrier tail if they ever change shape
            _state["nc"] = _build_module(lean_tail=False)
    return _state["nc"]


def kernel(x, weight_inputs, perm_mat):
    from concourse.bass_utils import run_bass_kernel_spmd

    x = np.ascontiguousarray(np.asarray(x, dtype=np.float32))
    w = np.asarray(weight_inputs, dtype=np.float32)
    pm = np.asarray(perm_mat)
    assert x.shape == (B, DIM), x.shape

    ni = w.shape[2]
    w0, w1 = w[0, 0], w[0, 1]
    c = (w0 + w1) * w0 * w1  # f32 elementwise, matches reference precision
    m = np.zeros((DIM, DIM), dtype=np.float32)
    m[pm[:ni], pm[ni:]] = c

    template = np.empty((P, BLOB_W), dtype=np.float32)
    template[:, 256:384] = m[0:128, 0:128]
    template[:, 384:512] = m[0:128, 128:256]
    template[:, 512:640] = m[128:256, 128:256]
    template[:, 640] = 1.0

    shards = x.reshape(NCORES, ROWS, DIM)
    in_maps = []
    for i in range(NCORES):
        blob = template.copy()
        blob[:, 0:128] = shards[i][:, 0:128].T
        blob[:, 128:256] = shards[i][:, 128:256].T
        in_maps.append({"blob": blob})

    try:
        r = run_bass_kernel_spmd(_get_nc(), in_maps, core_ids=list(range(NCORES)))
    except ModuleNotFoundError:
        # BASS_TRACE set but this container lacks the axon NTFF profile
        # hook (antenv.axon_hooks) — rerun untraced.
        os.environ["BASS_NEVER_TRACE"] = "1"
        r = run_bass_kernel_spmd(_get_nc(), in_maps, core_ids=list(range(NCORES)))
    _state["last_results"] = r
    out = np.concatenate([res["out"].reshape(ROWS, 1) for res in r.results], axis=0)
    return out


# revision 15
# speedup vs baseline: 1.0464x; 1.0464x over previous
"""Trainium2 Bass kernel for nn_AtomicFunction (pairwise-product quadratic form).

Reference math:
    out[b] = sum_k (w0[k]+w1[k]) * w0[k] * w1[k] * x[b, i_k] * x[b, j_k]
where (i_k, j_k) enumerate all C(256,2)=32640 pairs i<j (perm_mat) and
w = weight_inputs[0].  This is a quadratic form: with the [256,256] upper-
triangular coefficient matrix M[i_k, j_k] = (w0+w1)*w0*w1,
    out[b] = sum_j (x @ M)[b, j] * x[b, j]
M depends only on the small replicated weights, so it is built on host.

Sharding: data-parallel over B=1024 across 8 cores (128 rows each); M and
the ones-column are replicated.

The kernel works in transposed space (x is shipped as x^T per shard) so no
on-device transposes are needed and the result lands as a contiguous
[1, 128] row:
    tT[j, b] = sum_i M[i, j] * x[b, i]       (3 matmuls; M block-triangular,
                                              the zero block M[128:,:128] is
                                              never shipped or multiplied)
    p[j, b]  = tT[j, b] * xT[j, b]           (2 DVE elementwise muls)
    out[b]   = sum_j p[j, b]                 (ones-vector matmuls -> [1,128])

All operands ride in one [128, 641] f32 blob split across the two HWDGE
rings (scalar ring: x^T halves; sync ring: M blocks + ones) so each core
does exactly two input DMAs and one 512-byte output DMA.
Blob columns: xT0 | xT1 | M00 | M01 | M11 | ones.
"""

import os

import numpy as np

B = 1024
DIM = 256
NCORES = 8
ROWS = B // NCORES  # 128
P = 128
BLOB_W = 2 * P + 3 * P + 1  # 641

_state: dict = {}


def _lean_drain_and_barrier(self, tick_clock, wait_clock):
    """Lean TileContext tail for a single top-level context: drain with the
    full global-clock sem waits (proving every engine retired and the
    output DMA completed), ONE all-engine barrier so every engine has
    synced before the sem/DGE reset, then the reset.  The stock tail adds
    a second all-engine barrier after the reset, which only protects sem
    reuse by sibling tile contexts — this module has none.  Saves ~0.26us
    per launch and passes the CoreSim race detector's sem-clear rule."""
    from concourse.vector_clock import ScopedClock

    drain_inst = self.nc.sync.drain()
    wait_clock.add_sem_waits(drain_inst.ins, ScopedClock({None: tick_clock.global_clock}))
    self.nc.all_engine_barrier()
    popped = self.nc._tile_sem_poison_stack.pop()
    assert popped is self._sem_poison
    self.nc.clear_and_free_semaphores(list(self.sems.allocated().values()))


def _build_module(lean_tail=True):
    import concourse.mybir as mybir
    import concourse.tile as tile
    from concourse import bacc

    f32 = mybir.dt.float32
    nc = bacc.Bacc("TRN2", num_devices=NCORES)
    blob_d = nc.dram_tensor("blob", [P, BLOB_W], f32, kind="ExternalInput").ap()
    o_d = nc.dram_tensor("out", [1, ROWS], f32, kind="ExternalOutput").ap()

    if lean_tail:
        _orig_drain = tile.TileContext._drain_and_barrier
        tile.TileContext._drain_and_barrier = _lean_drain_and_barrier
        try:
            _emit_tile_body(nc, tile, f32, blob_d, o_d)
        finally:
            tile.TileContext._drain_and_barrier = _orig_drain
        _drop_const_pool_memsets(nc)
    else:
        _emit_tile_body(nc, tile, f32, blob_d, o_d)

    nc.compile()
    return nc


def _drop_const_pool_memsets(nc):
    """Bass.__init__ memsets four constant tensors (const-float32-0.0 etc.)
    on the Pool engine before the start barrier; this kernel never reads
    them, but the barrier — and therefore the first input DMA — waits for
    all four (~0.37us).  Drop the memsets (only if nothing else references
    the const tensors) so the whole pipeline starts earlier."""
    for blk in nc.main_func.blocks:
        for ins in blk.instructions:
            if type(ins).__name__ != "InstMemset" and "const-" in str(ins.ins) + str(ins.outs):
                return  # a kernel instruction uses the const pool; keep memsets
    bb = nc.main_func.blocks[0]
    bb.instructions = [
        ins for ins in bb.instructions
        if not (type(ins).__name__ == "InstMemset" and "const-" in str(ins.outs))
    ]


def _emit_tile_body(nc, tile, f32, blob_d, o_d):
    with tile.TileContext(nc) as tc:
        with (
            tc.tile_pool(name="sbuf", bufs=1) as pool,
            tc.tile_pool(name="psum", bufs=1, space="PSUM") as psum,
        ):
            blob = pool.tile([P, BLOB_W], f32)
            nc.scalar.dma_start(blob[:, 0:256], blob_d[:, 0:256])
            nc.sync.dma_start(blob[:, 256:BLOB_W], blob_d[:, 256:BLOB_W])
            xT0 = blob[:, 0:128]
            xT1 = blob[:, 128:256]
            M00 = blob[:, 256:384]
            M01 = blob[:, 384:512]
            M11 = blob[:, 512:640]
            ones = blob[:, 640:641]

            # tT = M^T-blocks @ xT, PSUM f32 accumulation
            tL = psum.tile([P, P], f32, tag="tL")
            nc.tensor.matmul(tL[:], M00, xT0, start=True, stop=True)
            tR = psum.tile([P, P], f32, tag="tR")
            nc.tensor.matmul(tR[:], M01, xT0, start=True, stop=False)
            nc.tensor.matmul(tR[:], M11, xT1, start=False, stop=True)

            # p = tT * xT elementwise
            p0 = pool.tile([P, P], f32, tag="p0")
            nc.vector.tensor_mul(out=p0[:], in0=tL[:], in1=xT0)
            p1 = pool.tile([P, P], f32, tag="p1")
            nc.vector.tensor_mul(out=p1[:], in0=tR[:], in1=xT1)

            # partition-dim reduction via ones-vector matmul -> [1, 128]
            op = psum.tile([1, P], f32, tag="op")
            nc.tensor.matmul(op[:], ones, p0[:], start=True, stop=False)
            nc.tensor.matmul(op[:], ones, p1[:], start=False, stop=True)
            resT = pool.tile([1, P], f32)
            nc.vector.tensor_copy(out=resT[:], in_=op[:])
            nc.sync.dma_start(o_d[:], resT[:])


def _get_nc():
    if "nc" not in _state:
        try:
            _state["nc"] = _build_module(lean_tail=True)
        except Exception:
            # the lean tail touches TileContext internals; fall back to the
            # stock drain+barrier tail if they ever change shape
            _state["nc"] = _build_module(lean_tail=False)
    return _state["nc"]


def kernel(x, weight_inputs, perm_mat):
    from concourse.bass_utils import run_bass_kernel_spmd

    x = np.ascontiguousarray(np.asarray(x, dtype=np.float32))
    w = np.asarray(weight_inputs, dtype=np.float32)
    pm = np.asarray(perm_mat)
    assert x.shape == (B, DIM), x.shape

    ni = w.shape[2]
    w0, w1 = w[0, 0], w[0, 1]
    c = (w0 + w1) * w0 * w1  # f32 elementwise, matches reference precision
    m = np.zeros((DIM, DIM), dtype=np.float32)
    m[pm[:ni], pm[ni:]] = c

    template = np.empty((P, BLOB_W), dtype=np.float32)
    template[:, 256:384] = m[0:128, 0:128]
    template[:, 384:512] = m[0:128, 128:256]
    template[:, 512:640] = m[128:256, 128:256]
    template[:, 640] = 1.0

    shards = x.reshape(NCORES, ROWS, DIM)
    in_maps = []
    for i in range(NCORES):
        blob = template.copy()
        blob[:, 0:128] = shards[i][:, 0:128].T
        blob[:, 128:256] = shards[i][:, 128:256].T
        in_maps.append({"blob": blob})

    try:
        r = run_bass_kernel_spmd(_get_nc(), in_maps, core_ids=list(range(NCORES)))
    except ModuleNotFoundError:
        # BASS_TRACE set but this container lacks the axon NTFF profile
        # hook (antenv.axon_hooks) — rerun untraced.
        os.environ["BASS_NEVER_TRACE"] = "1"
        r = run_bass_kernel_spmd(_get_nc(), in_maps, core_ids=list(range(NCORES)))
    _state["last_results"] = r
    out = np.concatenate([res["out"].reshape(ROWS, 1) for res in r.results], axis=0)
    return out
